# revision 1
# baseline (speedup 1.0000x reference)
"""Trainium2 Bass kernel for nn_DCT_Features (dense_cnn).

Math: everything before the LeakyReLU is linear, so the whole module
(3D DCT-II -> mean over dct bins -> per-subwindow full-volume Conv3d)
collapses to one GEMM per subwindow:

  out[b, s*128+k] = LeakyReLU( sum_{n,phi} x[b, s, n, phi] * Weff[s, phi, k] + conv_b[s, k] )

with the mean's 1/2 folded into
  Weff[s, (t,h,w), k] = 0.5 * sum_{f,g,j} conv_w[s,k,f,g,j] Ct[f,t] Ch[g,h] Cw[j,w]

Sharding: pure data parallel over batch, 8 cores x 512 rows; Weff/bias
replicated. The kernel is DMA-bound (all transfers serialize on HBM at
~360 B/ns), so precision is chosen per tensor to minimize bytes within
the 2e-2 error gate: x in fp8 e3m4 (4 mantissa bits; range +-15.5 covers
the N(0,1) input; measured rel err 1.4e-2), Weff/bias in bf16, output in
bf16. Host-side marshaling lays each core's shard out feature-major
([s, kt, p, n, b]) and converts dtype; no input arithmetic on host.

Per core: DMA x tile (fp8) -> matmul accumulate in fp32 PSUM (kout on
partitions, batch on free; the two dct bins contract against the same
weight tile via two matmuls, except a few k-tiles per chunk whose bins
are presummed on DVE to keep PE comfortably under the DMA roofline;
bias applied via a K=1 matmul against a memset ones row) -> exact
2-op LeakyReLU on DVE -> DMA out in bf16 (still [kout, batch]; host
upcasts + un-transposes while gathering the 8 shards).
"""

import os
from contextlib import ExitStack

import numpy as np
import ml_dtypes

import concourse.bass as bass
import concourse.tile as tile
from concourse import bacc, mybir
from concourse.bass_utils import run_bass_kernel_spmd

# Static problem config (hardcoded per contract)
B_FULL = 4096
N_CORES = 8
B_CORE = B_FULL // N_CORES      # 512 batch rows per core
N_SW = 2                        # subwindows
DCT_NBINS = 2
NDCT = 32                       # freqs per subwindow
H = W = 8
KF = NDCT * H * W               # 2048 contraction dim per subwindow per bin
KT = KF // 128                  # 16 k-tiles
KOUT = 128                      # output channels per subwindow
SLOPE = 0.001

W_COLS = N_SW * KT * KOUT       # 4096 weight columns
BIAS_COLS = N_SW * KOUT         # bias block first: row 0 of cols [0, 256)
W0 = BIAS_COLS                  # weight columns start here
WB_COLS = W_COLS + BIAS_COLS    # 4352

_CACHE = {}
LAST_RESULT = None


def _dct_mat(N):
    n = np.arange(N)
    k = np.arange(N)[:, None]
    return 2.0 * np.cos(np.pi * (2 * n + 1) * k / (2 * N))  # [k, n], float64


def _fold_weights(conv_w, conv_b):
    """Fold DCT matrices + mean into the conv weights (float64 host math)."""
    cw = np.asarray(conv_w, np.float64)          # [s, k, f, g, j]
    Ct = _dct_mat(NDCT)                          # [f, t]
    Ch = _dct_mat(H)                             # [g, h]
    Cw = _dct_mat(W)                             # [j, w]
    we = np.einsum("skfgj,ft,gh,jw->sthwk", cw, Ct, Ch, Cw) * 0.5
    we = we.reshape(N_SW, KF, KOUT)              # [s, phi, k]
    # SBUF layout: w_sb[p, (s*KT+kt)*128 + k] = we[s, kt*128+p, k];
    # bias rides in row 0 of the trailing 256 columns.
    wb = np.zeros((128, WB_COLS), np.float64)
    wb[:, W0:] = (
        we.reshape(N_SW, KT, 128, KOUT).transpose(2, 0, 1, 3).reshape(128, W_COLS)
    )
    wb[0, :BIAS_COLS] = np.asarray(conv_b, np.float64).reshape(-1)
    return np.ascontiguousarray(wb.astype(ml_dtypes.bfloat16))


def _shard_x(x):
    """Marshal x into per-core feature-major fp8(e3m4) tiles.

    Returns per-core arrays of shape [N_SW*KT*128, DCT_NBINS*B_CORE] where
    row (s*KT+kt)*128+p, column n*B_CORE+b holds x[c*B_CORE+b, f] with
    f = s*4096 + n*2048 + kt*128 + p.
    """
    X = np.asarray(x, np.float32).reshape(B_FULL, N_SW * DCT_NBINS * KF)
    shards = []
    for c in range(N_CORES):
        v = X[c * B_CORE : (c + 1) * B_CORE].reshape(B_CORE, N_SW, DCT_NBINS, KT, 128)
        p = v.transpose(1, 3, 4, 2, 0)  # [s, kt, p, n, b]
        shards.append(
            np.ascontiguousarray(p)
            .reshape(N_SW * KT * 128, DCT_NBINS * B_CORE)
            .astype(ml_dtypes.float8_e3m4)
        )
    return shards


CHUNK_KT = 4  # k-tiles per x DMA (0.5 MiB fp8 transfers, near HBM-rate)


def _chunk_plan(s):
    """(kt_start, n_kt) DMA chunks for subwindow s. Large chunks for DMA
    efficiency; the last-processed subwindow tapers so less serial work
    trails the final DMA (shorter kernel tail). The first chunk is small
    so PE can start working early (it is a near co-bottleneck)."""
    if s == 0:
        return [(0, 2), (2, 2), (4, 4), (8, 4), (12, 4)]
    return [(0, 2), (2, 2), (4, 2), (6, 2), (8, 2), (10, 2), (12, 2), (14, 1), (15, 1)]


def _presum_this(s, kt):
    """k-tiles whose dct bins are presummed on DVE (1 matmul instead of 2):
    offloads PE work to the mostly idle DVE so PE finishes with the DMA
    stream instead of draining a backlog after it. The tail k-tiles of each
    subwindow stay on PE so nothing after the last x chunk waits on the
    serial DVE queue."""
    return kt % 2 == 1 and kt < (14 if s == 0 else 12)


def _build_program():
    nc = bacc.Bacc("TRN2", target_bir_lowering=False, debug=False, num_devices=N_CORES)
    f32 = mybir.dt.float32
    bf16 = mybir.dt.bfloat16
    fp8 = mybir.dt.float8e3
    x_ap = nc.dram_tensor(
        "x", [N_SW * KT * 128, DCT_NBINS * B_CORE], fp8, kind="ExternalInput"
    ).ap()
    w_ap = nc.dram_tensor("w", [128, WB_COLS], bf16, kind="ExternalInput").ap()
    # output stays transposed [s*128+k, b]; host upcasts + un-transposes
    out_ap = nc.dram_tensor("out", [N_SW * KOUT, B_CORE], bf16, kind="ExternalOutput").ap()

    with tile.TileContext(nc) as tc, ExitStack() as ctx:
        const = ctx.enter_context(tc.tile_pool(name="const", bufs=1))
        x_pool = ctx.enter_context(tc.tile_pool(name="xp", bufs=11))
        y_pool = ctx.enter_context(tc.tile_pool(name="yp", bufs=13))
        osb_pool = ctx.enter_context(tc.tile_pool(name="osb", bufs=8))
        pout_pool = ctx.enter_context(tc.tile_pool(name="pout", bufs=2, space="PSUM"))

        ones = const.tile([1, B_CORE], bf16)
        nc.gpsimd.memset(ones[:], 1.0)
        # Weight staging: [bias | s0 kt0-3] small and first on SP HWDGE (so
        # PE can start the moment the first x chunk lands); s0 kt4-15 via
        # Pool/SWDGE (descriptor gen on the idle GPSIMD engine, off the
        # busy shared HWDGE); s1 weights as four 4kt pieces, each issued on
        # the same queue as — and directly before — the x chunk that first
        # needs it, so the DMA-engine FIFO delivers them in need order
        # instead of letting early-ready bulk weights delay mid-stream x
        # chunks.
        wsplit = W0 + 4 * KOUT
        w_first = const.tile([128, wsplit], bf16)
        w_s0b = const.tile([128, (KT - 4) * KOUT], bf16)
        w_s1g = [
            const.tile([128, 8 * KOUT], bf16, name=f"w_s1g{i}") for i in range(2)
        ]
        nc.sync.dma_start(out=w_first[:], in_=w_ap[:, 0:wsplit])
        nc.gpsimd.dma_start(out=w_s0b[:], in_=w_ap[:, wsplit : W0 + KT * KOUT])

        def w_lhsT(s, kt):
            if s == 0 and kt < 4:
                return w_first[:, bass.ds(W0 + kt * KOUT, KOUT)]
            if s == 0:
                return w_s0b[:, bass.ds((kt - 4) * KOUT, KOUT)]
            return w_s1g[kt // 8][:, bass.ds((kt % 8) * KOUT, KOUT)]

        x_re = x_ap.rearrange("(t p) f -> p t f", p=128)  # [128, 32, 1024]

        psums = []
        for s in range(N_SW):
            psum_out = pout_pool.tile([KOUT, B_CORE], f32)
            psums.append(psum_out)
            # bias via K=1 matmul against the ones row: starts the psum
            # accumulation group and keeps bias-add off the epilogue.
            nc.tensor.matmul(
                psum_out[:],
                lhsT=w_first[0:1, bass.ds(s * KOUT, KOUT)],
                rhs=ones[:],
                start=True,
                stop=False,
            )
            for g, (kt0, nkt) in enumerate(_chunk_plan(s)):
                xab = x_pool.tile([128, CHUNK_KT, DCT_NBINS * B_CORE], fp8)
                # alternate the two HWDGE queues (SP / ACT) for pipelined
                # descriptor generation while transfers serialize on HBM
                dma_eng = nc.scalar if (s * 4 + g) % 2 == 0 else nc.sync
                if s == 1 and kt0 % 8 == 0:
                    wg = kt0 // 8
                    dma_eng.dma_start(
                        out=w_s1g[wg][:],
                        in_=w_ap[:, bass.ds(W0 + (KT + wg * 8) * KOUT, 8 * KOUT)],
                    )
                dma_eng.dma_start(
                    out=xab[:, 0:nkt, :], in_=x_re[:, bass.ds(s * KT + kt0, nkt), :]
                )
                kts = [kt0 + j for j in range(nkt)]
                presum = [kt for kt in kts if _presum_this(s, kt)]
                # DVE presum adds first so their y tiles are in flight...
                ys = {}
                for kt in presum:
                    y = y_pool.tile([128, B_CORE], bf16)
                    nc.vector.tensor_add(
                        y[:],
                        xab[:, kt - kt0, 0:B_CORE],
                        xab[:, kt - kt0, B_CORE:],
                    )
                    ys[kt] = y
                # ...then the direct matmuls (PE executes in order: a matmul
                # waiting on DVE must not block ready ones behind it), and
                # the presummed k-tiles' matmuls last.
                mms = [(kt, n) for kt in kts if kt not in ys for n in range(DCT_NBINS)]
                mms += [(kt, None) for kt in presum]
                for i, (kt, n) in enumerate(mms):
                    rhs = ys[kt][:] if n is None else xab[:, kt - kt0, bass.ts(n, B_CORE)]
                    nc.tensor.matmul(
                        psum_out[:],
                        lhsT=w_lhsT(s, kt),
                        rhs=rhs,
                        start=False,
                        stop=(g == len(_chunk_plan(s)) - 1 and i == len(mms) - 1),
                    )

        # Epilogues, emitted AFTER all x/w DMA issues so no ACT/DVE op ever
        # sits ahead of an x chunk in a queue's program order, and each out
        # DMA emitted IMMEDIATELY after its producer (consumers wait on
        # per-engine counters, so a later unrelated op on the producer
        # engine would falsely delay the out DMA). s=0 (deep in the
        # stream's shadow) uses the one-instruction ACT Lrelu; s=1 is the
        # kernel tail and uses the exact max(y, SLOPE*y) form on DVE.
        for s in range(N_SW):
            for h in range(2):
                hb = bass.ts(h, B_CORE // 2)
                osb = osb_pool.tile(
                    [KOUT, B_CORE // 2], bf16, tag="osb", name=f"osb_{s}_{h}"
                )
                if s == 0:
                    nc.scalar.activation(
                        osb[:],
                        psums[s][:, hb],
                        mybir.ActivationFunctionType.Lrelu,
                        alpha=SLOPE,
                    )
                else:
                    tl = osb_pool.tile(
                        [KOUT, B_CORE // 2], f32, tag="tl", name=f"tl_{s}_{h}"
                    )
                    nc.vector.tensor_scalar_mul(tl[:], psums[s][:, hb], SLOPE)
                    nc.vector.tensor_max(osb[:], psums[s][:, hb], tl[:])
                eng = nc.sync if h == 0 else nc.scalar
                eng.dma_start(out=out_ap[bass.ts(s, KOUT), hb], in_=osb[:])

    nc.compile()
    return nc


def _get_program():
    if "nc" not in _CACHE:
        _CACHE["nc"] = _build_program()
    return _CACHE["nc"]


def kernel(x, conv_w, conv_b):
    global LAST_RESULT
    shards = _shard_x(x)
    wb_host = _fold_weights(conv_w, conv_b)

    nc = _get_program()
    in_maps = [{"x": shards[c], "w": wb_host} for c in range(N_CORES)]
    trace = bool(int(os.environ.get("DCT_TRACE", "0")))
    res = run_bass_kernel_spmd(nc, in_maps, list(range(N_CORES)), trace=trace)
    LAST_RESULT = res
    # per-core output is [s*128+k, b] bf16; upcast + un-transpose during gather
    out = np.concatenate(
        [
            np.ascontiguousarray(np.asarray(res.results[c]["out"], np.float32).T)
            for c in range(N_CORES)
        ],
        axis=0,
    )
    return out



# revision 38
# speedup vs baseline: 1.1131x; 1.1131x over previous
"""Trainium2 Bass kernel for nn_DCT_Features (dense_cnn).

Math: everything before the LeakyReLU is linear, so the whole module
(3D DCT-II -> mean over dct bins -> per-subwindow full-volume Conv3d)
collapses to one GEMM per subwindow:

  out[b, s*128+k] = LeakyReLU( sum_{n,phi} x[b, s, n, phi] * Weff[s, phi, k] + conv_b[s, k] )

with the mean's 1/2 folded into
  Weff[s, (t,h,w), k] = 0.5 * sum_{f,g,j} conv_w[s,k,f,g,j] Ct[f,t] Ch[g,h] Cw[j,w]

Sharding: pure data parallel over batch, 8 cores x 512 rows; Weff/bias
replicated. All DMA transfers serialize on the device's DMA engines at
~360 B/ns, so the kernel is built around one gap-free DMA stream:

- x in fp8 e3m4 with error-feedback across the two dct bins (the bins
  contract against the same weights, so only the bin-SUM's quantization
  error matters; quantizing bin1 against bin0's rounding residual cuts
  x's noise ~sqrt(2)).
- Weff in fp8 e3m4 as well (halves weight bytes; measured end-to-end
  rel err 1.66e-2 vs the 2e-2 gate), bias bf16, output bf16.

Per core: weights + x stream in on SP/ACT (HWDGE) and Pool (SWDGE)
queues sized so descriptor generation hides under transfers; PE runs
one matmul per (k-tile, bin) into fp32 PSUM (kout on partitions, batch
free), except ~11 k-tiles whose bins are presummed on the otherwise
idle DVE so PE tracks the stream rate. The epilogue is a single ACT
instruction per subwindow (LeakyReLU with per-partition bias fused via
the activation bias port); its act table is preloaded at t=0 by a dummy
activation, and PE's pstate ramp is burned off by warmup matmuls into a
scratch PSUM bank before real data lands. Output stays [s*128+k, b]
bf16; host upcasts + un-transposes while gathering the 8 shards.
"""

import os
from contextlib import ExitStack

import numpy as np
import ml_dtypes

import concourse.bass as bass
import concourse.tile as tile
from concourse import bacc, mybir
from concourse.bass_utils import run_bass_kernel_spmd

# Static problem config (hardcoded per contract)
B_FULL = 4096
N_CORES = 8
B_CORE = B_FULL // N_CORES      # 512 batch rows per core
N_SW = 2                        # subwindows
DCT_NBINS = 2
NDCT = 32                       # freqs per subwindow
H = W = 8
KF = NDCT * H * W               # 2048 contraction dim per subwindow per bin
KT = KF // 128                  # 16 k-tiles
KOUT = 128                      # output channels per subwindow
SLOPE = 0.001

W_COLS = N_SW * KT * KOUT       # 4096 weight columns
N_WARMUP = 6                    # PE pstate warmup matmuls
USE_WRITEBACK = bool(int(os.environ.get("DCT_WRITEBACK", "1")))

_CACHE = {}
LAST_RESULT = None


def _dct_mat(N):
    n = np.arange(N)
    k = np.arange(N)[:, None]
    return 2.0 * np.cos(np.pi * (2 * n + 1) * k / (2 * N))  # [k, n], float64


def _fold_weights(conv_w):
    """Fold DCT matrices + mean into the conv weights (float64 host math)."""
    cw = np.asarray(conv_w, np.float64)          # [s, k, f, g, j]
    Ct = _dct_mat(NDCT)                          # [f, t]
    Ch = _dct_mat(H)                             # [g, h]
    Cw = _dct_mat(W)                             # [j, w]
    we = np.einsum("skfgj,ft,gh,jw->sthwk", cw, Ct, Ch, Cw) * 0.5
    we = we.reshape(N_SW, KF, KOUT)              # [s, phi, k]
    # SBUF layout: w_sb[p, (s*KT+kt)*128 + k] = we[s, kt*128+p, k]
    wb = (
        we.reshape(N_SW, KT, 128, KOUT).transpose(2, 0, 1, 3).reshape(128, W_COLS)
    )
    return np.ascontiguousarray(wb.astype(ml_dtypes.float8_e3m4))


def _bias_host(conv_b):
    """bias_sb[k, s] = conv_b[s, k], bf16."""
    return np.ascontiguousarray(
        np.asarray(conv_b, np.float64).T.astype(ml_dtypes.bfloat16)
    )


def _shard_x(x):
    """Marshal x into per-core feature-major fp8(e3m4) tiles.

    Row (s*KT+kt)*128+p, column n*B_CORE+b holds the quantization of
    x[c*B_CORE+b, f] with f = s*4096 + n*2048 + kt*128 + p. Bin 1 is
    quantized with error feedback from bin 0 (the two bins contract
    against the same weights, so only their sum's error matters).
    """
    X = np.asarray(x, np.float32).reshape(B_FULL, N_SW * DCT_NBINS * KF)
    e3 = ml_dtypes.float8_e3m4
    shards = []
    for c in range(N_CORES):
        v = X[c * B_CORE : (c + 1) * B_CORE].reshape(B_CORE, N_SW, DCT_NBINS, KT, 128)
        p = np.ascontiguousarray(v.transpose(1, 3, 4, 2, 0))  # [s, kt, p, n, b] f32
        q = np.empty_like(p, dtype=e3)
        q0 = p[:, :, :, 0].astype(e3)
        q[:, :, :, 0] = q0
        resid = p[:, :, :, 0] - q0.astype(np.float32)
        q[:, :, :, 1] = (p[:, :, :, 1] + resid).astype(e3)
        shards.append(q.reshape(N_SW * KT * 128, DCT_NBINS * B_CORE))
    return shards


# Chunk plan: (kt_start, n_kt, [presummed kts], bins) per subwindow.
# bins=None means both dct bins ride in one transfer; bins=(n,) streams a
# single bin of one k-tile (182ns transfer, one matmul). 4-ktile chunks
# keep HWDGE gen (~630ns) well under each transfer (1456ns); ~2 presums
# per chunk keep PE's and DVE's per-chunk demand under the arrival rate.
# The final k-tiles stream as tiny direct chunks (no DVE presum chain
# between the last transfer and the epilogue), ending with two single-bin
# transfers so only ONE matmul trails the final x semaphore.
# Junction rule: chunk c's PE work must fit the window to the NEXT chunk's
# semaphore (= next chunk's transfer time), so chunk sizes taper
# 4kt -> 2kt -> 1kt toward the end: a coarse chunk right before a fine one
# strands PE with a backlog it can never recover, since direct consumption
# (426ns/kt) outpaces arrival (364ns/kt). Presummed k-tiles' matmuls are
# DEFERRED one chunk slot so they never wait on the 594ns DVE add latency,
# and presums alternate with direct k-tiles in the 1kt tail to keep both
# PE's and DVE's slot work under the window.
_PLAN = {
    0: [
        (0, 4, [], None),
        (4, 4, [5, 7], None),
        (8, 2, [9], None),
        (10, 2, [11], None),
        (12, 2, [13], None),
        (14, 2, [15], None),
    ],
    1: [
        (0, 2, [1], None),
        (2, 2, [3], None),
        (4, 2, [5], None),
        (6, 2, [7], None),
        (8, 2, [9], None),
        (10, 2, [11], None),
        (12, 2, [], None),
        (14, 1, [], None),
        (15, 1, [], (0,)),
        (15, 1, [], (1,)),
    ],
}


def _build_program():
    nc = bacc.Bacc(
        "TRN2",
        target_bir_lowering=False,
        debug=False,
        num_devices=N_CORES,
        num_swdge_queues=2,
    )
    f32 = mybir.dt.float32
    bf16 = mybir.dt.bfloat16
    fp8 = mybir.dt.float8e3
    x_ap = nc.dram_tensor(
        "x", [N_SW * KT * 128, DCT_NBINS * B_CORE], fp8, kind="ExternalInput"
    ).ap()
    w_ap = nc.dram_tensor("w", [128, W_COLS], fp8, kind="ExternalInput").ap()
    b_ap = nc.dram_tensor("bias", [128, N_SW], bf16, kind="ExternalInput").ap()
    # output stays [k, s, b]; host upcasts + un-transposes during gather
    out_ap = nc.dram_tensor(
        "out", [KOUT, N_SW, B_CORE], bf16, kind="ExternalOutput"
    ).ap()

    with tile.TileContext(nc) as tc, ExitStack() as ctx:
        const = ctx.enter_context(tc.tile_pool(name="const", bufs=1))
        x_pool = ctx.enter_context(tc.tile_pool(name="xp", bufs=16))
        y_pool = ctx.enter_context(tc.tile_pool(name="yp", bufs=13))
        osb_pool = ctx.enter_context(tc.tile_pool(name="osb", bufs=2))
        pout_pool = ctx.enter_context(tc.tile_pool(name="pout", bufs=1, space="PSUM"))

        # Warmup operands, memset on DVE so SP/ACT/Pool stay clear for DMA
        # issue.
        warm_a = const.tile([1, KOUT], bf16, name="warm_a")
        warm_b = const.tile([1, B_CORE], bf16, name="warm_b")
        scratch = const.tile([1, KOUT], bf16, name="scratch")
        nc.vector.memset(warm_a[:], 1.0)
        nc.vector.memset(warm_b[:], 1.0)
        # Dummy activation emitted FIRST on ACT: its implicit LoadActFuncSet
        # (1.3us) runs on the ACT engine at t~=0 while the ACT sequencer moves
        # straight on to x-chunk DMA descriptor generation; the real epilogue
        # Lrelu then needs no table load. The dummy itself parks in the ACT
        # engine wait queue until the DVE memset lands - it never blocks the
        # SEQ.
        nc.scalar.activation(
            scratch[:],
            warm_a[:],
            mybir.ActivationFunctionType.Lrelu,
            bias=warm_a[:, 0:1],
            alpha=SLOPE,
        )

        x_re = x_ap.rearrange("(t p) f -> p t f", p=128)  # [128, 32, 1024]

        # All x chunks go on the ACT queue IN ORDER: DMA transfers are granted
        # in request order, and both PE and DVE consume chunks in program
        # order, so a single in-order queue avoids arrival scrambling (which
        # cascades into in-order-engine stalls). ACT rather than SP because
        # the Tile preamble parks ~650ns of setup on SP, delaying its first
        # descriptor gen. The first x chunk is issued before anything else so
        # its transfer leads the stream.
        def issue_chunk(s, kt0, nkt, bins):
            nb = DCT_NBINS if bins is None else len(bins)
            cols = nb * B_CORE
            name = f"x_{s}_{kt0}" + ("" if bins is None else f"_b{bins[0]}")
            xab = x_pool.tile([128, nkt, cols], fp8, tag="x", name=name)
            src = x_re[:, bass.ds(s * KT + kt0, nkt), :]
            if bins is not None:
                src = x_re[
                    :, bass.ds(s * KT + kt0, nkt), bass.ds(bins[0] * B_CORE, B_CORE)
                ]
            nc.sync.dma_start(out=xab[:, 0:nkt, :], in_=src)
            return xab

        # Weight/bias staging: the first s0 k-tiles lead the stream on SP
        # (tiny 182ns transfer) so PE's first matmuls wait only on x; the
        # rest via Pool/SWDGE (descriptor gen on the idle Pool engine, off
        # the shared HWDGE, requests interleaving into the FIFO well before
        # the k-tiles that need them).
        w_sb = const.tile([128, W_COLS], fp8, name="w_sb")
        bias_sb = const.tile([128, N_SW], bf16, name="bias_sb")
        c0 = _PLAN[0][0]
        wsplit = c0[1] * KOUT
        xab0 = issue_chunk(0, c0[0], c0[1], c0[3])
        nc.sync.dma_start(out=w_sb[:, 0:wsplit], in_=w_ap[:, 0:wsplit])
        nc.gpsimd.dma_start(out=bias_sb[:], in_=b_ap[:, :])
        nc.gpsimd.dma_start(
            out=w_sb[:, wsplit : KT * KOUT], in_=w_ap[:, wsplit : KT * KOUT]
        )
        nc.gpsimd.dma_start(out=w_sb[:, KT * KOUT :], in_=w_ap[:, KT * KOUT :])

        # Triggered output writeback: descriptors for the [k, s, b] output
        # tile are PRE-GENERATED on SWDGE queue 1 (kv_writeback prepare_only
        # defers the data dependency to the trigger), so after the final
        # activation the output transfer starts ~70ns later instead of paying
        # the ~1.3us HWDGE descriptor-gen + dispatch latency.
        osb = osb_pool.tile([KOUT, N_SW, B_CORE], bf16, name="osb")
        if USE_WRITEBACK:
            wb_idx = const.tile([128, 1], mybir.dt.int32, name="wb_idx")
            nc.gpsimd.memset(wb_idx[:], 0)
            wb_sem = nc.alloc_semaphore("out_wb")
            out4 = out_ap.rearrange("(o k) s b -> o k s b", o=1)
            osb4 = osb[:].rearrange("k s (c b) -> k s c b", c=1)
            nc.gpsimd.kv_writeback(
                out4, osb4, wb_idx[:], prepare_only=True, sem=wb_sem, queue_num=1
            )

        def w_lhsT(s, kt):
            return w_sb[:, bass.ds((s * KT + kt) * KOUT, KOUT)]

        # PE pstate warmup: K=1 matmuls into a scratch PSUM bank that is
        # never read. By the time real data lands (~3.8us) the PE has been
        # continuously busy >3us and runs at 2.4GHz from the first real mm.
        psum_warm = pout_pool.tile([KOUT, B_CORE], f32, name="psum_warm")
        for i in range(N_WARMUP):
            nc.tensor.matmul(
                psum_warm[:],
                lhsT=warm_a[:],
                rhs=warm_b[:],
                start=(i == 0),
                stop=(i == N_WARMUP - 1),
            )

        psums = []
        xtiles = {}
        for s in range(N_SW):
            psum_out = pout_pool.tile([KOUT, B_CORE], f32, name=f"psum_{s}")
            psums.append(psum_out)
            # Build the per-chunk matmul schedule: direct matmuls run in
            # their own chunk's slot; presummed k-tiles' matmuls DEFER one
            # slot so the DVE add (594ns after the chunk sem) is always done
            # before PE reaches them. The final chunk's pending presums
            # append inline at the end.
            sched = []  # (chunk_idx, [(kt, bin_or_None), ...])
            pending = []
            for ci, (kt0, nkt, presum, bins) in enumerate(_PLAN[s]):
                kts = list(range(kt0, kt0 + nkt))
                nbins = range(DCT_NBINS) if bins is None else range(len(bins))
                mms = list(pending)
                mms += [(kt, n) for kt in kts if kt not in presum for n in nbins]
                pending = [(kt, None) for kt in presum]
                sched.append(mms)
            sched[-1] = sched[-1] + pending

            total = sum(len(m) for m in sched)
            done = 0
            first = True
            ys = {}
            for ci, (kt0, nkt, presum, bins) in enumerate(_PLAN[s]):
                if s == 0 and ci == 0:
                    xab = xab0
                else:
                    xab = issue_chunk(s, kt0, nkt, bins)
                xabs = {kt: (xab, kt - kt0) for kt in range(kt0, kt0 + nkt)}
                xtiles.update({(s, kt): v for kt, v in xabs.items()})
                # DVE presum adds for THIS chunk (their matmuls come later)
                for kt in presum:
                    y = y_pool.tile([128, B_CORE], bf16, tag="y", name=f"y_{s}_{kt}")
                    nc.vector.tensor_add(
                        y[:],
                        xab[:, kt - kt0, 0:B_CORE],
                        xab[:, kt - kt0, B_CORE:],
                    )
                    ys[kt] = y
                for kt, n in sched[ci]:
                    if n is None:
                        rhs = ys[kt][:]
                    else:
                        t, idx = xtiles[(s, kt)]
                        rhs = t[:, idx, bass.ts(n, B_CORE)]
                    done += 1
                    nc.tensor.matmul(
                        psum_out[:],
                        lhsT=w_lhsT(s, kt),
                        rhs=rhs,
                        start=first,
                        stop=(done == total),
                    )
                    first = False

        # Epilogues, emitted AFTER all x/w DMA issues. One ACT instruction
        # per subwindow: osb[:, s, :] = Lrelu(psum + bias[:, s]); then one
        # trigger fires the pre-generated output descriptors (Tile attaches
        # the RAW deps on both activations to the trigger automatically).
        for s in range(N_SW):
            nc.scalar.activation(
                osb[:, s, :],
                psums[s][:],
                mybir.ActivationFunctionType.Lrelu,
                bias=bias_sb[:, bass.ds(s, 1)],
                alpha=SLOPE,
            )
        if USE_WRITEBACK:
            nc.gpsimd.trigger_dma(count=None, queue_num=1)
        else:
            nc.scalar.dma_start(out=out_ap[:, :, :], in_=osb[:])

    if USE_WRITEBACK:
        _patch_prep_completion_sem(nc)
    nc.compile()
    return nc


def _patch_prep_completion_sem(nc):
    """Point the kv_writeback prep's DMA-completion update at Tile's DMASW
    lane semaphore.

    Tile's wait pass makes downstream consumers (the exit barrier) wait on
    the DMASW lane it assigned to the prep, but the prep's OnUpdate[0] is
    the user-supplied sem= (walrus bakes OnUpdate[0] into the descriptors as
    the completion sem), so the lane sem would never fire. Rewrite
    OnUpdate[0] to the lane sem: the one DMASW sem that is waited on but
    never updated by any instruction.
    """
    import concourse.mybir as mb

    fn = nc.m.functions[0]
    updated, waited = set(), {}
    preps = []
    for b in fn.blocks:
        for i in b.instructions:
            si = i.sync_info
            if si is None:
                continue
            for u in si.on_update:
                updated.add(u.id)
            for w in si.on_wait:
                if (w.ant_name or "").startswith("DMASW"):
                    waited[w.id] = (w.ant_name, w.wait_value)
            if type(i).__name__ == "InstKVWritebackAnt":
                preps.append(i)
    missing = {sid: nv for sid, nv in waited.items() if sid not in updated}
    assert len(preps) == 1 and len(missing) == 1, (preps, missing, waited)
    (sid, (name, val)), (prep,) = missing.popitem(), preps
    si = prep.sync_info
    keep = [u for u in si.on_update if u.ant_name != "out_wb"]
    lane = mb.SyncUpdate(
        sync_type="semaphore",
        id=sid,
        ant_name=name,
        update_mode="sem-add-imm",
        update_value=16,
    )
    prep.sync_info = mb.SyncInfo(on_wait=list(si.on_wait), on_update=[lane] + keep)


def _get_program():
    if "nc" not in _CACHE:
        _CACHE["nc"] = _build_program()
    return _CACHE["nc"]


def kernel(x, conv_w, conv_b):
    global LAST_RESULT
    shards = _shard_x(x)
    w_host = _fold_weights(conv_w)
    b_host = _bias_host(conv_b)

    nc = _get_program()
    in_maps = [{"x": shards[c], "w": w_host, "bias": b_host} for c in range(N_CORES)]
    trace = bool(int(os.environ.get("DCT_TRACE", "0")))
    res = run_bass_kernel_spmd(nc, in_maps, list(range(N_CORES)), trace=trace)
    LAST_RESULT = res
    # per-core output is [k, s, b] bf16; upcast + un-transpose during gather
    out = np.concatenate(
        [
            np.ascontiguousarray(
                np.asarray(res.results[c]["out"], np.float32)
                .transpose(2, 1, 0)
                .reshape(B_CORE, N_SW * KOUT)
            )
            for c in range(N_CORES)
        ],
        axis=0,
    )
    return out


# revision 92
# speedup vs baseline: 1.1625x; 1.0444x over previous
"""Trainium2 Bass kernel for nn_DCT_Features (dense_cnn).

Math: everything before the LeakyReLU is linear, so the whole module
(3D DCT-II -> mean over dct bins -> per-subwindow full-volume Conv3d)
collapses to one GEMM per subwindow:

  out[b, s*128+k] = LeakyReLU( sum_{n,phi} x[b, s, n, phi] * Weff[s, phi, k] + conv_b[s, k] )

with the mean's 1/2 folded into
  Weff[s, (t,h,w), k] = 0.5 * sum_{f,g,j} conv_w[s,k,f,g,j] Ct[f,t] Ch[g,h] Cw[j,w]

Sharding: pure data parallel over batch, 8 cores x 512 rows; Weff/bias
replicated. All DMA transfers serialize on the device's DMA engines at
~360 B/ns, so the kernel is built around one gap-free DMA stream:

- x in fp8 e3m4 with error-feedback across the two dct bins (the bins
  contract against the same weights, so only the bin-SUM's quantization
  error matters; quantizing bin1 against bin0's rounding residual cuts
  x's noise ~sqrt(2)).
- Weff in fp8 e3m4 as well (halves weight bytes; measured end-to-end
  rel err 1.66e-2 vs the 2e-2 gate), bias bf16, output bf16.

Per core: the x chunks stream IN PROGRAM ORDER on the SP queue (a single
in-order queue keeps arrival order aligned with the in-order PE/DVE
consumers; DMA grants are FIFO by request), while the fp8 weights + bias
ride the Pool/SWDGE queue whose descriptor gen runs off the shared
HWDGE. PE runs one matmul per (k-tile, bin) into fp32 PSUM (kout on
partitions, batch free), except 13 k-tiles whose bins are presummed on
the otherwise idle DVE; presummed matmuls are deferred one chunk slot so
they never wait on the 594ns DVE add. Chunk sizes taper 4kt -> 2kt ->
1kt -> single-bin toward the stream tail so PE's work per semaphore
window never exceeds the arrival window (direct consumption, 426ns/kt,
outpaces arrival at 364ns/kt; chunks below ~1kt granularity instead trip
the 8-deep DMAHW semaphore-lane recycling, which makes DMA i wait on
completion of DMA i-8). The epilogue is a single ACT instruction per
subwindow (LeakyReLU with per-partition bias fused via the activation
bias port); its act table is preloaded at t=0 by a dummy activation, and
PE's pstate ramp is burned off by warmup matmuls into a scratch PSUM
bank before real data lands. Output stays [k, s, b] bf16; host upcasts +
un-transposes while gathering the 8 shards.

(A SWDGE prepare/trigger output writeback that pre-generates the output
descriptors saves another ~1.9us in the cost model but returns NaN on
real hardware through the PJRT flow, so it is disabled; see
USE_WRITEBACK.)
"""

import os
from contextlib import ExitStack

import numpy as np
import ml_dtypes

import concourse.bass as bass
import concourse.tile as tile
from concourse import bacc, mybir
from concourse.bass_utils import run_bass_kernel_spmd

# Static problem config (hardcoded per contract)
B_FULL = 4096
N_CORES = 8
B_CORE = B_FULL // N_CORES      # 512 batch rows per core
N_SW = 2                        # subwindows
DCT_NBINS = 2
NDCT = 32                       # freqs per subwindow
H = W = 8
KF = NDCT * H * W               # 2048 contraction dim per subwindow per bin
KT = KF // 128                  # 16 k-tiles
KOUT = 128                      # output channels per subwindow
SLOPE = 0.001

W_COLS = N_SW * KT * KOUT       # 4096 weight columns
N_WARMUP = 6                    # PE pstate warmup matmuls
# The SWDGE prepare/trigger writeback path is fast in the cost model but
# produces NaN on real hardware via the PJRT flow (the triggered-descriptor
# ucode appears unsupported there), so it stays off.
USE_WRITEBACK = bool(int(os.environ.get("DCT_WRITEBACK", "0")))

_CACHE = {}
LAST_RESULT = None


def _dct_mat(N):
    n = np.arange(N)
    k = np.arange(N)[:, None]
    return 2.0 * np.cos(np.pi * (2 * n + 1) * k / (2 * N))  # [k, n], float64


def _fold_weights(conv_w):
    """Fold DCT matrices + mean into the conv weights (float64 host math)."""
    cw = np.asarray(conv_w, np.float64)          # [s, k, f, g, j]
    Ct = _dct_mat(NDCT)                          # [f, t]
    Ch = _dct_mat(H)                             # [g, h]
    Cw = _dct_mat(W)                             # [j, w]
    we = np.einsum("skfgj,ft,gh,jw->sthwk", cw, Ct, Ch, Cw) * 0.5
    we = we.reshape(N_SW, KF, KOUT)              # [s, phi, k]
    # SBUF layout: w_sb[p, (s*KT+kt)*128 + k] = we[s, kt*128+p, k]
    wb = (
        we.reshape(N_SW, KT, 128, KOUT).transpose(2, 0, 1, 3).reshape(128, W_COLS)
    )
    return np.ascontiguousarray(wb.astype(ml_dtypes.float8_e3m4))


def _bias_host(conv_b):
    """bias_sb[k, s] = conv_b[s, k], bf16."""
    return np.ascontiguousarray(
        np.asarray(conv_b, np.float64).T.astype(ml_dtypes.bfloat16)
    )


def _shard_x(x):
    """Marshal x into per-core feature-major fp8(e3m4) tiles.

    Row (s*KT+kt)*128+p, column n*B_CORE+b holds the quantization of
    x[c*B_CORE+b, f] with f = s*4096 + n*2048 + kt*128 + p. Bin 1 is
    quantized with error feedback from bin 0 (the two bins contract
    against the same weights, so only their sum's error matters).
    """
    X = np.asarray(x, np.float32).reshape(B_FULL, N_SW * DCT_NBINS * KF)
    e3 = ml_dtypes.float8_e3m4
    shards = []
    for c in range(N_CORES):
        v = X[c * B_CORE : (c + 1) * B_CORE].reshape(B_CORE, N_SW, DCT_NBINS, KT, 128)
        p = np.ascontiguousarray(v.transpose(1, 3, 4, 2, 0))  # [s, kt, p, n, b] f32
        q = np.empty_like(p, dtype=e3)
        q0 = p[:, :, :, 0].astype(e3)
        q[:, :, :, 0] = q0
        resid = p[:, :, :, 0] - q0.astype(np.float32)
        q[:, :, :, 1] = (p[:, :, :, 1] + resid).astype(e3)
        shards.append(q.reshape(N_SW * KT * 128, DCT_NBINS * B_CORE))
    return shards


# Chunk plan: (kt_start, n_kt, [presummed kts], bins) per subwindow.
# bins=None means both dct bins ride in one transfer; bins=(n,) streams a
# single bin of one k-tile (182ns transfer, one matmul). 4-ktile chunks
# keep HWDGE gen (~630ns) well under each transfer (1456ns); ~2 presums
# per chunk keep PE's and DVE's per-chunk demand under the arrival rate.
# The final k-tiles stream as tiny direct chunks (no DVE presum chain
# between the last transfer and the epilogue), ending with two single-bin
# transfers so only ONE matmul trails the final x semaphore.
# Junction rule: chunk c's PE work must fit the window to the NEXT chunk's
# semaphore (= next chunk's transfer time), so chunk sizes taper
# 4kt -> 2kt -> 1kt toward the end: a coarse chunk right before a fine one
# strands PE with a backlog it can never recover, since direct consumption
# (426ns/kt) outpaces arrival (364ns/kt). Presummed k-tiles' matmuls are
# DEFERRED one chunk slot so they never wait on the 594ns DVE add latency,
# and presums alternate with direct k-tiles in the 1kt tail to keep both
# PE's and DVE's slot work under the window.
_PLAN = {
    0: [
        (0, 4, [], None),
        (4, 4, [5, 7], None),
        (8, 2, [9], None),
        (10, 2, [11], None),
        (12, 2, [13], None),
        (14, 2, [15], None),
    ],
    1: [
        (0, 2, [1], None),
        (2, 2, [3], None),
        (4, 2, [5], None),
        (6, 1, [], None),
        (7, 1, [7], None),
        (8, 1, [], None),
        (9, 1, [9], None),
        (10, 1, [], None),
        (11, 1, [11], None),
        (12, 1, [], None),
        (13, 1, [13], None),
        (15, 1, [], (0,)),
        (15, 1, [], (1,)),
        (14, 1, [], None),
    ],
}


def _build_program():
    nc = bacc.Bacc(
        "TRN2",
        target_bir_lowering=False,
        debug=False,
        num_devices=N_CORES,
    )
    f32 = mybir.dt.float32
    bf16 = mybir.dt.bfloat16
    fp8 = mybir.dt.float8e3
    x_ap = nc.dram_tensor(
        "x", [N_SW * KT * 128, DCT_NBINS * B_CORE], fp8, kind="ExternalInput"
    ).ap()
    w_ap = nc.dram_tensor("w", [128, W_COLS], fp8, kind="ExternalInput").ap()
    b_ap = nc.dram_tensor("bias", [128, N_SW], bf16, kind="ExternalInput").ap()
    # output stays [k, s, b]; host upcasts + un-transposes during gather
    out_ap = nc.dram_tensor(
        "out", [KOUT, N_SW, B_CORE], bf16, kind="ExternalOutput"
    ).ap()

    with tile.TileContext(nc) as tc, ExitStack() as ctx:
        const = ctx.enter_context(tc.tile_pool(name="const", bufs=1))
        x_pool = ctx.enter_context(tc.tile_pool(name="xp", bufs=11))
        y_pool = ctx.enter_context(tc.tile_pool(name="yp", bufs=13))
        osb_pool = ctx.enter_context(tc.tile_pool(name="osb", bufs=2))
        pout_pool = ctx.enter_context(tc.tile_pool(name="pout", bufs=1, space="PSUM"))

        # Warmup operands, memset on DVE so SP/ACT/Pool stay clear for DMA
        # issue.
        warm_a = const.tile([1, KOUT], bf16, name="warm_a")
        warm_b = const.tile([1, B_CORE], bf16, name="warm_b")
        scratch = const.tile([1, KOUT], bf16, name="scratch")
        nc.vector.memset(warm_a[:], 1.0)
        nc.vector.memset(warm_b[:], 1.0)
        # Dummy activation emitted FIRST on ACT: its implicit LoadActFuncSet
        # (1.3us) runs on the ACT engine at t~=0 while the ACT sequencer moves
        # straight on to x-chunk DMA descriptor generation; the real epilogue
        # Lrelu then needs no table load. The dummy itself parks in the ACT
        # engine wait queue until the DVE memset lands - it never blocks the
        # SEQ.
        nc.scalar.activation(
            scratch[:],
            warm_a[:],
            mybir.ActivationFunctionType.Lrelu,
            bias=warm_a[:, 0:1],
            alpha=SLOPE,
        )

        x_re = x_ap.rearrange("(t p) f -> p t f", p=128)  # [128, 32, 1024]

        # All x chunks go on the ACT queue IN ORDER: DMA transfers are granted
        # in request order, and both PE and DVE consume chunks in program
        # order, so a single in-order queue avoids arrival scrambling (which
        # cascades into in-order-engine stalls). ACT rather than SP because
        # the Tile preamble parks ~650ns of setup on SP, delaying its first
        # descriptor gen. The first x chunk is issued before anything else so
        # its transfer leads the stream.
        def issue_chunk(s, kt0, nkt, bins, eng=None):
            nb = DCT_NBINS if bins is None else len(bins)
            cols = nb * B_CORE
            name = f"x_{s}_{kt0}" + ("" if bins is None else f"_b{bins[0]}")
            xab = x_pool.tile([128, nkt, cols], fp8, tag="x", name=name)
            src = x_re[:, bass.ds(s * KT + kt0, nkt), :]
            if bins is not None:
                src = x_re[
                    :, bass.ds(s * KT + kt0, nkt), bass.ds(bins[0] * B_CORE, B_CORE)
                ]
            (eng or nc.sync).dma_start(out=xab[:, 0:nkt, :], in_=src)
            return xab

        # Weight/bias staging: the first s0 k-tiles lead the stream on SP
        # (tiny 182ns transfer) so PE's first matmuls wait only on x; the
        # rest via Pool/SWDGE (descriptor gen on the idle Pool engine, off
        # the shared HWDGE, requests interleaving into the FIFO well before
        # the k-tiles that need them).
        w_sb = const.tile([128, W_COLS], fp8, name="w_sb")
        bias_sb = const.tile([128, N_SW], bf16, name="bias_sb")
        c0 = _PLAN[0][0]
        wsplit = c0[1] * KOUT
        xab0 = issue_chunk(0, c0[0], c0[1], c0[3])
        nc.sync.dma_start(out=w_sb[:, 0:wsplit], in_=w_ap[:, 0:wsplit])
        nc.sync.dma_start(out=bias_sb[:], in_=b_ap[:, :])
        nc.gpsimd.dma_start(
            out=w_sb[:, wsplit : KT * KOUT], in_=w_ap[:, wsplit : KT * KOUT]
        )
        nc.gpsimd.dma_start(out=w_sb[:, KT * KOUT :], in_=w_ap[:, KT * KOUT :])

        # Triggered output writeback: descriptors for the [k, s, b] output
        # tile are PRE-GENERATED on SWDGE queue 1 (kv_writeback prepare_only
        # defers the data dependency to the trigger), so after the final
        # activation the output transfer starts ~70ns later instead of paying
        # the ~1.3us HWDGE descriptor-gen + dispatch latency.
        osb = osb_pool.tile([KOUT, N_SW, B_CORE], bf16, name="osb")
        if USE_WRITEBACK:
            wb_idx = const.tile([128, 1], mybir.dt.int32, name="wb_idx")
            nc.gpsimd.memset(wb_idx[:], 0)
            wb_sem = nc.alloc_semaphore("out_wb")
            out4 = out_ap.rearrange("(o k) s b -> o k s b", o=1)
            osb4 = osb[:].rearrange("k s (c b) -> k s c b", c=1)
            nc.gpsimd.kv_writeback(
                out4, osb4, wb_idx[:], prepare_only=True, sem=wb_sem, queue_num=0
            )

        def w_lhsT(s, kt):
            return w_sb[:, bass.ds((s * KT + kt) * KOUT, KOUT)]

        # PE pstate warmup: K=1 matmuls into a scratch PSUM bank that is
        # never read. By the time real data lands (~3.8us) the PE has been
        # continuously busy >3us and runs at 2.4GHz from the first real mm.
        psum_warm = pout_pool.tile([KOUT, B_CORE], f32, name="psum_warm")
        for i in range(N_WARMUP):
            nc.tensor.matmul(
                psum_warm[:],
                lhsT=warm_a[:],
                rhs=warm_b[:],
                start=(i == 0),
                stop=(i == N_WARMUP - 1),
            )

        psums = []
        xtiles = {}
        for s in range(N_SW):
            psum_out = pout_pool.tile([KOUT, B_CORE], f32, name=f"psum_{s}")
            psums.append(psum_out)
            # Build the per-chunk matmul schedule: direct matmuls run in
            # their own chunk's slot; presummed k-tiles' matmuls DEFER one
            # slot so the DVE add (594ns after the chunk sem) is always done
            # before PE reaches them. The final chunk's pending presums
            # append inline at the end.
            sched = []  # (chunk_idx, [(kt, bin_or_None), ...])
            pending = []
            for ci, (kt0, nkt, presum, bins) in enumerate(_PLAN[s]):
                kts = list(range(kt0, kt0 + nkt))
                nbins = range(DCT_NBINS) if bins is None else range(len(bins))
                mms = list(pending)
                mms += [(kt, n) for kt in kts if kt not in presum for n in nbins]
                pending = [(kt, None) for kt in presum]
                sched.append(mms)
            sched[-1] = sched[-1] + pending

            total = sum(len(m) for m in sched)
            done = 0
            first = True
            ys = {}
            for ci, (kt0, nkt, presum, bins) in enumerate(_PLAN[s]):
                if s == 0 and ci == 0:
                    xab = xab0
                else:
                    # s1's first two chunks ride Pool/SWDGE: their descriptor
                    # gen runs off the shared HWDGE, shortening the SP issue
                    # chain so the tail chunks stop arriving gen-bound. They
                    # land early/out-of-order, which is harmless (psum
                    # accumulation is commutative and semaphore-gated).
                    pool_eng = None  # all x on SP: any off-queue chunk displaces the in-order stream
                    xab = issue_chunk(s, kt0, nkt, bins, eng=pool_eng)
                xabs = {kt: (xab, kt - kt0) for kt in range(kt0, kt0 + nkt)}
                xtiles.update({(s, kt): v for kt, v in xabs.items()})
                # DVE presum adds for THIS chunk (their matmuls come later)
                for kt in presum:
                    y = y_pool.tile([128, B_CORE], bf16, tag="y", name=f"y_{s}_{kt}")
                    nc.vector.tensor_add(
                        y[:],
                        xab[:, kt - kt0, 0:B_CORE],
                        xab[:, kt - kt0, B_CORE:],
                    )
                    ys[kt] = y
                for kt, n in sched[ci]:
                    if n is None:
                        rhs = ys[kt][:]
                    else:
                        t, idx = xtiles[(s, kt)]
                        rhs = t[:, idx, bass.ts(n, B_CORE)]
                    done += 1
                    nc.tensor.matmul(
                        psum_out[:],
                        lhsT=w_lhsT(s, kt),
                        rhs=rhs,
                        start=first,
                        stop=(done == total),
                    )
                    first = False

        # Epilogues, emitted AFTER all x/w DMA issues. One ACT instruction
        # per subwindow: osb[:, s, :] = Lrelu(psum + bias[:, s]); then one
        # trigger fires the pre-generated output descriptors (Tile attaches
        # the RAW deps on both activations to the trigger automatically).
        for s in range(N_SW):
            nc.scalar.activation(
                osb[:, s, :],
                psums[s][:],
                mybir.ActivationFunctionType.Lrelu,
                bias=bias_sb[:, bass.ds(s, 1)],
                alpha=SLOPE,
            )
        if USE_WRITEBACK:
            nc.gpsimd.trigger_dma(count=None, queue_num=0)
        else:
            # Split outs: s0's rides the stream mid-flight (its HWDGE issue
            # latency hides under the x stream; at worst it inserts one
            # 364ns transfer before the final chunks), s1's is the tail.
            nc.sync.dma_start(out=out_ap[:, 0, :], in_=osb[:, 0, :])
            nc.sync.dma_start(out=out_ap[:, 1, :], in_=osb[:, 1, :])

    if USE_WRITEBACK:
        _patch_prep_completion_sem(nc)
    nc.compile()
    return nc


def _patch_prep_completion_sem(nc):
    """Point the kv_writeback prep's DMA-completion update at Tile's DMASW
    lane semaphore.

    Tile's wait pass makes downstream consumers (the exit barrier) wait on
    the DMASW lane it assigned to the prep, but the prep's OnUpdate[0] is
    the user-supplied sem= (walrus bakes OnUpdate[0] into the descriptors as
    the completion sem), so the lane sem would never fire. Rewrite
    OnUpdate[0] to the lane sem: the one DMASW sem that is waited on but
    never updated by any instruction.
    """
    import concourse.mybir as mb

    fn = nc.m.functions[0]
    updated, waited = set(), {}
    preps = []
    for b in fn.blocks:
        for i in b.instructions:
            si = i.sync_info
            if si is None:
                continue
            for u in si.on_update:
                updated.add(u.id)
            for w in si.on_wait:
                if (w.ant_name or "").startswith("DMASW"):
                    waited[w.id] = (w.ant_name, w.wait_value)
            if type(i).__name__ == "InstKVWritebackAnt":
                preps.append(i)
    missing = {sid: nv for sid, nv in waited.items() if sid not in updated}
    assert len(preps) == 1 and len(missing) == 1, (preps, missing, waited)
    (sid, (name, val)), (prep,) = missing.popitem(), preps
    si = prep.sync_info
    keep = [u for u in si.on_update if u.ant_name != "out_wb"]
    lane = mb.SyncUpdate(
        sync_type="semaphore",
        id=sid,
        ant_name=name,
        update_mode="sem-add-imm",
        update_value=16,
    )
    prep.sync_info = mb.SyncInfo(on_wait=list(si.on_wait), on_update=[lane] + keep)


def _get_program():
    if "nc" not in _CACHE:
        _CACHE["nc"] = _build_program()
    return _CACHE["nc"]


def kernel(x, conv_w, conv_b):
    global LAST_RESULT
    shards = _shard_x(x)
    w_host = _fold_weights(conv_w)
    b_host = _bias_host(conv_b)

    nc = _get_program()
    in_maps = [{"x": shards[c], "w": w_host, "bias": b_host} for c in range(N_CORES)]
    trace = bool(int(os.environ.get("DCT_TRACE", "0")))
    res = run_bass_kernel_spmd(nc, in_maps, list(range(N_CORES)), trace=trace)
    LAST_RESULT = res
    # per-core output is [k, s, b] bf16; upcast + un-transpose during gather
    out = np.concatenate(
        [
            np.ascontiguousarray(
                np.asarray(res.results[c]["out"], np.float32)
                .transpose(2, 1, 0)
                .reshape(B_CORE, N_SW * KOUT)
            )
            for c in range(N_CORES)
        ],
        axis=0,
    )
    return out
